# revision 5
# baseline (speedup 1.0000x reference)
"""Cross-attention kernel for Trainium2, data-parallel over batch on 8 cores.

Per core (one batch element):
  Q = x @ Wq + bq ; K = e @ Wk + bk ; V = e @ Wv + bv
  out = softmax(Q K^T / 8) @ V

Fast path (biases all zero, which is what setup_inputs produces) uses the
association S = x (Wq Wk^T) e^T and is organized to minimize PE stationary
(weight) loads, which cost ~140ns each on HW and dominate once streaming is
accounted for:

  - All input transposes (xT, eT, wqT, wkT) are done with ZERO PE work:
    the DMA load uses a permuted DRAM-side access pattern (keeping
    128B-contiguous runs) that block-permutes 32x32 tiles into place, then
    one DVE StreamTranspose per [128, S] tile performs the within-block
    32x32 transposes, with a fused f32->f16 downcast.  This removes the
    384 PE transpose matmuls (~275ns each on HW) of the previous version.
  - G = Wq @ Wk^T           [64 ldweights, 128 matmuls]
  - HT = G^T @ xT           [64 ldweights (stationary g-chunk reused over
                             the 4 sq chunks), 256 matmuls]
  - ST = eT^T @ HT fused with V = eT^T @ Wv: each eT stationary chunk is
    loaded ONCE and used by 6 matmuls (4 score chunks + 2 V halves)
                            [128 ldweights, 768 matmuls]
  - PT = exp(ST/8) (no max subtraction: |s/8| < ~25 fits fp32/bf16)
  - out = (PT^T @ V) * 1/(PT^T @ 1)  row sums from the same stationary
                            [256 ldweights, 768 matmuls]

Total: 512 stationary loads vs 2304 in the naive form.  PSUM banks are
assigned so no two back-to-back matmuls hit the same bank (~80ns stall).

Numerics: fp16 projections + bf16 probs/V gives ~2e-3 rel L2 error vs the
fp32 reference.
"""

import numpy as np

import concourse.bacc as bacc
import concourse.bass as bass
import concourse.mybir as mybir
import concourse.tile as tile
from concourse.bass_utils import run_bass_kernel_spmd
from concourse.masks import make_identity

P = 128
D = 1024
ND = D // P  # 8 d tiles
SQ = 2048
NSQ = SQ // P  # 16
SKV = 2048
NSKV = SKV // P  # 16
NC = SQ // 512  # 4 strips of 512 along s
N_CORES = 8

F32 = mybir.dt.float32
F16 = mybir.dt.float16
BF16 = mybir.dt.bfloat16
AF = mybir.ActivationFunctionType


def _load_w16(nc, ld_pool, w16_pool, w_dram, ld_tag="ldw", w_tag="w16"):
    tiles = []
    for dit in range(ND):
        wl = ld_pool.tile([P, D], F32, name=f"wl_{w_dram.name}_{dit}", tag=ld_tag)
        nc.sync.dma_start(wl[:], w_dram.ap()[dit * P : (dit + 1) * P, :])
        w16t = w16_pool.tile([P, D], F16, name=f"w16_{w_dram.name}_{dit}", tag=w_tag)
        nc.vector.tensor_copy(w16t[:], wl[:])
        tiles.append(w16t)
    return tiles


def build(reps=1, fast=False):
    nc = bacc.Bacc("TRN2", target_bir_lowering=False, debug=False)
    x = nc.declare_dram_parameter("x", [SQ, D], F32, isOutput=False)
    e = nc.declare_dram_parameter("e", [SKV, D], F32, isOutput=False)
    wq = nc.declare_dram_parameter("wq", [D, D], F32, isOutput=False)
    wk = nc.declare_dram_parameter("wk", [D, D], F32, isOutput=False)
    wv = nc.declare_dram_parameter("wv", [D, D], F32, isOutput=False)
    bq = nc.declare_dram_parameter("bq", [D], F32, isOutput=False)
    bk = nc.declare_dram_parameter("bk", [D], F32, isOutput=False)
    bv = nc.declare_dram_parameter("bv", [D], F32, isOutput=False)
    out = nc.declare_dram_parameter("out", [SQ, D], F32, isOutput=True)

    with tile.TileContext(nc) as tc:
        for _rep in range(reps):
            if fast:
                _emit_body_fast(nc, tc, x, e, wq, wk, wv, out)
            else:
                _emit_body(nc, tc, x, e, wq, wk, wv, bq, bk, bv, out)

    nc.compile()
    return nc


def _emit_body_fast(nc, tc, x, e, wq, wk, wv, out):
    # ---- left-stack pools, allocated in reverse death order (LIFO) ----
    const = tc.alloc_tile_pool(name="const", bufs=1, side="left")
    ht_pool = tc.alloc_tile_pool(name="ht", bufs=ND, side="left")
    et_pool = tc.alloc_tile_pool(name="et", bufs=ND, side="left")
    wv16p = tc.alloc_tile_pool(name="wv16", bufs=ND, side="left")
    xt_pool = tc.alloc_tile_pool(name="xt", bufs=ND, side="left")
    g16_pool = tc.alloc_tile_pool(name="g16", bufs=ND, side="left")
    stg = tc.alloc_tile_pool(name="stg", bufs=2, side="left")
    stg16 = tc.alloc_tile_pool(name="stg16", bufs=2, side="left")
    ldn = tc.alloc_tile_pool(name="ldn", bufs=2, side="left")
    wqT_pool = tc.alloc_tile_pool(name="wqT", bufs=ND, side="left")
    wkT_pool = tc.alloc_tile_pool(name="wkT", bufs=ND, side="left")

    ones_col = const.tile([P, 1], BF16, tag="ones_col")
    nc.gpsimd.memset(ones_col[:], 1.0)

    hT = [ht_pool.tile([P, SQ], F16, name=f"hT{d}", tag="hT") for d in range(ND)]
    eT = [et_pool.tile([P, SKV], F16, name=f"eT{d}", tag="eT") for d in range(ND)]
    xT = [xt_pool.tile([P, SQ], F16, name=f"xT{d}", tag="xT") for d in range(ND)]
    wqT = [wqT_pool.tile([P, D], F16, name=f"wqT{d}", tag="wqT") for d in range(ND)]
    wkT = [wkT_pool.tile([P, D], F16, name=f"wkT{d}", tag="wkT") for d in range(ND)]

    def scat_tile(src_dram, s_rows, dst, di, tag, dma_eng, cast_dve):
        """dst[p, c] = f16(src[c, 128*di + p]).

        The DMA places src[32g+a, col0+32be+b] at y[32be+a, 32g+b] (inner
        b-run stays 128B-contiguous in DRAM), a cast narrows to f16, then
        one DVE StreamTranspose flips each 32x32 block in place.
        ACT does the cast for early tiles; DVE for late ones (keeps the
        in-order ACT stream free for psum evictions).
        """
        g = s_rows // 32
        y = stg.tile([P, s_rows], F32, name=f"y_{tag}{di}", tag="stg")
        for be in range(4):
            col0 = 128 * di + 32 * be
            src = src_dram.ap()[:, col0 : col0 + 32].rearrange(
                "(g a) b -> a g b", g=g, a=32
            )
            ydst = y[32 * be : 32 * (be + 1), :].rearrange(
                "p (g b) -> p g b", g=g, b=32
            )
            dma_eng.dma_start(ydst, src)
        y16 = stg16.tile([P, s_rows], F16, name=f"y16_{tag}{di}", tag="stg16")
        if cast_dve:
            nc.vector.tensor_copy(y16[:], y[:])
        else:
            nc.scalar.activation(y16[:], y[:], AF.Identity)
        nc.vector.transpose(dst[:], y16[:])

    # ---- loads: wq/wk interleaved per tile (G consumes k-pairs in order),
    # then x; both weights on the SP queue, x/e on the ACT queue ----
    for di in range(ND):
        scat_tile(wq, D, wqT[di], di, "wq", nc.sync, cast_dve=False)
        scat_tile(wk, D, wkT[di], di, "wk", nc.sync, cast_dve=False)
    for di in range(ND):
        scat_tile(x, SQ, xT[di], di, "x", nc.scalar, cast_dve=False)
    for di in range(ND):
        scat_tile(e, SKV, eT[di], di, "e", nc.scalar, cast_dve=True)

    # ---- G = Wq @ Wk^T: two psum-limited passes of 4 m-tiles; within a
    # pass the k loop is outermost so weight tiles are consumed as the DMA
    # delivers them ----
    ps_g = tc.alloc_tile_pool(name="ps_g", bufs=4, space="PSUM")
    g16 = [g16_pool.tile([P, D], F16, name=f"g16_{m}", tag="g16") for m in range(ND)]
    for half in range(2):
        ms = range(4 * half, 4 * half + 4)
        psm = {m: ps_g.tile([P, D], F32, name=f"psg{m}", tag="psg") for m in ms}
        for k in range(ND):
            for m in ms:
                lhsT = wqT[k][:, m * P : (m + 1) * P]
                for h in range(2):
                    nc.tensor.matmul(
                        psm[m][:, h * 512 : (h + 1) * 512],
                        lhsT,
                        wkT[k][:, h * 512 : (h + 1) * 512],
                        start=(k == 0),
                        stop=(k == ND - 1),
                    )
        for m in ms:
            nc.scalar.activation(g16[m][:], psm[m][:], AF.Identity)
    ps_g.release()
    wkT_pool.release()
    wqT_pool.release()

    # ---- HT = G^T @ xT: four passes of dot-pairs (8 psum banks); each
    # stationary g-chunk is held across the 4 sq chunks ----
    ps_ht = tc.alloc_tile_pool(name="ps_ht", bufs=2, space="PSUM")
    for pair in range(4):
        dots = (2 * pair, 2 * pair + 1)
        pst = {
            dot: ps_ht.tile([P, SQ], F32, name=f"ps_ht{dot}", tag="ps_ht")
            for dot in dots
        }
        for dit in range(ND):
            for dot in dots:
                lhsT = g16[dit][:, dot * P : (dot + 1) * P]
                for c in range(NC):
                    nc.tensor.matmul(
                        pst[dot][:, c * 512 : (c + 1) * 512],
                        lhsT,
                        xT[dit][:, c * 512 : (c + 1) * 512],
                        start=(dit == 0),
                        stop=(dit == ND - 1),
                    )
        for dot in dots:
            nc.scalar.activation(hT[dot][:], pst[dot][:], AF.Identity)
    ps_ht.release()

    # ---- wv: natural layout + cast, loaded after G/HT so its ACT casts
    # sit behind the G/HT evictions in the in-order ACT stream ----
    wv16 = []
    for dit in range(ND):
        wl = ldn.tile([P, D], F32, name=f"wvl{dit}", tag="ldn")
        nc.sync.dma_start(wl[:], wv.ap()[dit * P : (dit + 1) * P, :])
        w16t = wv16p.tile([P, D], F16, name=f"wv16_{dit}", tag="wv16")
        nc.scalar.activation(w16t[:], wl[:], AF.Identity)
        wv16.append(w16t)

    ldn.release()
    stg16.release()
    stg.release()
    g16_pool.release()
    xt_pool.release()

    # ---- fused ST + V: per (kt, dit) the eT stationary chunk is loaded
    # once and drives 4 score matmuls + 2 V matmuls (6 distinct banks) ----
    v_pool = tc.alloc_tile_pool(name="v", bufs=NSKV, side="right")
    pt_pool = tc.alloc_tile_pool(name="pt", bufs=NSKV, side="right")
    vt = [v_pool.tile([P, D], BF16, name=f"v{t}", tag="v") for t in range(NSKV)]
    pT = [pt_pool.tile([P, SQ], BF16, name=f"pT{t}", tag="pT") for t in range(NSKV)]

    ps_st = tc.alloc_tile_pool(name="ps_st", bufs=2, space="PSUM")
    ps_v = tc.alloc_tile_pool(name="ps_v", bufs=2, space="PSUM")
    for kt in range(NSKV):
        pa = ps_st.tile([P, 1024], F32, name="pss_a", tag="pss")
        pb = ps_st.tile([P, 1024], F32, name="pss_b", tag="pss")
        psv = ps_v.tile([P, D], F32, name="psv", tag="psv")
        st_tiles = (pa, pa, pb, pb)
        for dit in range(ND):
            lhsT = eT[dit][:, kt * P : (kt + 1) * P]
            first = dit == 0
            last = dit == ND - 1
            for c in range(NC):
                nc.tensor.matmul(
                    st_tiles[c][:, (c % 2) * 512 : (c % 2 + 1) * 512],
                    lhsT,
                    hT[dit][:, c * 512 : (c + 1) * 512],
                    start=first,
                    stop=last,
                )
            for h in range(2):
                nc.tensor.matmul(
                    psv[:, h * 512 : (h + 1) * 512],
                    lhsT,
                    wv16[dit][:, h * 512 : (h + 1) * 512],
                    start=first,
                    stop=last,
                )
        nc.scalar.activation(pT[kt][:, 0:1024], pa[:], AF.Exp, scale=0.125)
        nc.scalar.activation(pT[kt][:, 1024:2048], pb[:], AF.Exp, scale=0.125)
        nc.scalar.activation(vt[kt][:], psv[:], AF.Identity)
    ps_v.release()
    ps_st.release()
    wv16p.release()
    et_pool.release()
    ht_pool.release()

    # ---- PV: out = (PT^T @ V) / (PT^T @ 1) ----
    outp = tc.alloc_tile_pool(name="outp", bufs=3, side="right")
    small = tc.alloc_tile_pool(name="small", bufs=4, side="right")
    ps_pv = tc.alloc_tile_pool(name="ps_pv", bufs=2, space="PSUM")
    ps_sum = tc.alloc_tile_pool(name="ps_sum", bufs=2, space="PSUM")
    for sqt in range(NSQ):
        pso = ps_pv.tile([P, D], F32, name="pso", tag="pso")
        psum_s = ps_sum.tile([P, 1], F32, name="psum_s", tag="psum_s")
        for kt in range(NSKV):
            lhsT = pT[kt][:, sqt * P : (sqt + 1) * P]
            first = kt == 0
            last = kt == NSKV - 1
            for h in range(2):
                nc.tensor.matmul(
                    pso[:, h * 512 : (h + 1) * 512],
                    lhsT,
                    vt[kt][:, h * 512 : (h + 1) * 512],
                    start=first,
                    stop=last,
                )
            nc.tensor.matmul(psum_s[:], lhsT, ones_col[:], start=first, stop=last)
        recip = small.tile([P, 1], F32, name="recip", tag="recip")
        nc.vector.reciprocal(recip[:], psum_s[:])
        ot = outp.tile([P, D], F32, name="ot", tag="ot")
        nc.vector.tensor_scalar_mul(ot[:], pso[:], recip[:])
        nc.scalar.dma_start(out.ap()[sqt * P : (sqt + 1) * P, :], ot[:])

    ps_sum.release()
    ps_pv.release()
    small.release()
    outp.release()
    pt_pool.release()
    v_pool.release()
    const.release()


def _emit_body(nc, tc, x, e, wq, wk, wv, bq, bk, bv, out):
    """General path (nonzero biases): projections with bias via ACT."""
    # ---- left-stack pools (released LIFO) ----
    const = tc.alloc_tile_pool(name="const", bufs=1, side="left")
    qt_pool = tc.alloc_tile_pool(name="qt", bufs=ND, side="left")
    kt_pool = tc.alloc_tile_pool(name="kt", bufs=ND, side="left")
    w16_pool = tc.alloc_tile_pool(name="w16", bufs=16, side="left")
    et_pool = tc.alloc_tile_pool(name="et", bufs=ND, side="left")
    ldW = tc.alloc_tile_pool(name="ldW", bufs=3, side="left")
    ldE = tc.alloc_tile_pool(name="ldE", bufs=4, side="left")
    xl16_pool = tc.alloc_tile_pool(name="xl16", bufs=4, side="left")
    ps_proj = tc.alloc_tile_pool(name="ps_proj", bufs=4, space="PSUM")
    ps_tr = tc.alloc_tile_pool(name="ps_tr", bufs=4, space="PSUM")

    identity = const.tile([P, P], F16, tag="ident")
    make_identity(nc, identity[:])
    ones_row = const.tile([1, P], F16, tag="ones_row")
    nc.gpsimd.memset(ones_row[:], 1.0)
    ones_col = const.tile([P, 1], BF16, tag="ones_col")
    nc.gpsimd.memset(ones_col[:], 1.0)
    bqt = const.tile([P, ND], F32, tag="bqt")
    nc.sync.dma_start(bqt[:], bq.ap().rearrange("(t p) -> p t", p=P))
    bkt = const.tile([P, ND], F32, tag="bkt")
    nc.sync.dma_start(bkt[:], bk.ap().rearrange("(t p) -> p t", p=P))
    bvl = ldW.tile([1, D], F32, tag="ldw")
    nc.sync.dma_start(bvl[:], bv.ap().rearrange("(a n) -> a n", a=1))
    bv16 = const.tile([1, D], F16, tag="bv16")
    nc.vector.tensor_copy(bv16[:], bvl[:])

    def transpose_group(ld_tiles, dst_write, tag):
        l16 = []
        for j in range(4):
            t16 = xl16_pool.tile([P, D], F16, name=f"l16_{tag}_{j}", tag="l16")
            nc.vector.tensor_copy(t16[:], ld_tiles[j][:])
            l16.append(t16)
        for dit2 in range(0, ND, 2):
            psts = [
                ps_tr.tile([P, 512], F16, name=f"pst_{tag}{u}", tag="pst")
                for u in range(2)
            ]
            for j in range(4):
                for u in range(2):
                    nc.tensor.matmul(
                        psts[u][:, j * P : (j + 1) * P],
                        l16[j][:, (dit2 + u) * P : (dit2 + u + 1) * P],
                        identity[:],
                        is_transpose=True,
                        start=(j == 0),
                        stop=(j == 3),
                    )
            for u in range(2):
                dst_write(dit2 + u, psts[u])

    def project_chunk(w16, rhs_of_dit, dst_tiles, bias_cols, c):
        for dot2 in range(0, ND, 2):
            psq = [
                ps_proj.tile([P, 512], F32, name=f"psq{u}", tag="psp")
                for u in range(2)
            ]
            for dit in range(ND):
                for u in range(2):
                    nc.tensor.matmul(
                        psq[u][:],
                        w16[dit][:, (dot2 + u) * P : (dot2 + u + 1) * P],
                        rhs_of_dit(dit),
                        start=(dit == 0),
                        stop=(dit == ND - 1),
                    )
            for u in range(2):
                nc.scalar.activation(
                    dst_tiles[dot2 + u][:, c * 512 : (c + 1) * 512],
                    psq[u][:],
                    AF.Identity,
                    bias=bias_cols[:, dot2 + u : dot2 + u + 1],
                )

    # ---- x -> xT chunks -> QT, interleaved per 512-chunk ----
    xtc_pool = tc.alloc_tile_pool(name="xtc", bufs=2 * ND, side="left")
    ldX = tc.alloc_tile_pool(name="ldX", bufs=4, side="left")

    def load_group(pool, src_dram, c, tag):
        tiles = []
        for j in range(4):
            st = c * 4 + j
            t = pool.tile([P, D], F32, name=f"{tag}{c}_{j}", tag=tag)
            nc.sync.dma_start(t[:], src_dram.ap()[st * P : (st + 1) * P, :])
            tiles.append(t)
        return tiles

    qT = [qt_pool.tile([P, SQ], F16, name=f"qT{d}", tag="qT") for d in range(ND)]
    xg = {0: load_group(ldX, x, 0, "ldx")}
    wq16 = _load_w16(nc, ldW, w16_pool, wq)
    eg = {0: load_group(ldE, e, 0, "lde")}
    for c in range(NC):
        if c + 1 < NC:
            xg[c + 1] = load_group(ldX, x, c + 1, "ldx")
        xtc = [
            xtc_pool.tile([P, 512], F16, name=f"xtc{c}_{d}", tag="xtc")
            for d in range(ND)
        ]

        def wr_x(dit, pst, xtc=xtc):
            nc.vector.tensor_copy(xtc[dit][:], pst[:])

        transpose_group(xg.pop(c), wr_x, "x")
        project_chunk(wq16, lambda dit, xtc=xtc: xtc[dit][:], qT, bqt, c)
    ldX.release()
    xtc_pool.release()

    # ---- e -> eT (kept resident) -> KT, interleaved per 512-chunk ----
    eT = [et_pool.tile([P, SKV], F16, name=f"eT{d}", tag="eT") for d in range(ND)]
    kT = [kt_pool.tile([P, SKV], F16, name=f"kT{d}", tag="kT") for d in range(ND)]
    wk16 = _load_w16(nc, ldW, w16_pool, wk)
    for c in range(NC):
        if c + 1 < NC:
            eg[c + 1] = load_group(ldE, e, c + 1, "lde")

        def wr_e(dit, pst, c=c):
            nc.vector.tensor_copy(eT[dit][:, c * 512 : (c + 1) * 512], pst[:])

        transpose_group(eg.pop(c), wr_e, "e")
        project_chunk(
            wk16,
            lambda dit, c=c: eT[dit][:, c * 512 : (c + 1) * 512],
            kT,
            bkt,
            c,
        )
    xl16_pool.release()
    ldE.release()

    # ---- Wv ; V ----
    wv16 = _load_w16(nc, ldW, w16_pool, wv)
    ldW.release()
    ps_tr.release()

    v_pool = tc.alloc_tile_pool(name="v", bufs=NSKV, side="right")
    vt = [v_pool.tile([P, D], BF16, name=f"v{t}", tag="v") for t in range(NSKV)]
    for kt_i in range(NSKV):
        ps_half = []
        for h in range(2):
            psv = ps_proj.tile([P, 512], F32, name=f"psv{h}", tag="psp")
            nc.tensor.matmul(
                psv[:],
                ones_row[:],
                bv16[:, h * 512 : (h + 1) * 512],
                start=True,
                stop=False,
            )
            ps_half.append(psv)
        for dit in range(ND):
            for h in range(2):
                nc.tensor.matmul(
                    ps_half[h][:],
                    eT[dit][:, kt_i * P : (kt_i + 1) * P],
                    wv16[dit][:, h * 512 : (h + 1) * 512],
                    start=False,
                    stop=(dit == ND - 1),
                )
        for h in range(2):
            nc.vector.tensor_copy(vt[kt_i][:, h * 512 : (h + 1) * 512], ps_half[h][:])

    ps_proj.release()
    et_pool.release()
    w16_pool.release()

    # ---- attention: ST+exp phase (full PT materialized), then PV phase ----
    pt_pool = tc.alloc_tile_pool(name="pt", bufs=NSKV, side="right")
    outp = tc.alloc_tile_pool(name="outp", bufs=3, side="right")
    small = tc.alloc_tile_pool(name="small", bufs=4, side="right")

    pT = [pt_pool.tile([P, SQ], BF16, name=f"pT{t}", tag="pT") for t in range(NSKV)]
    ps_st = tc.alloc_tile_pool(name="ps_st", bufs=2, space="PSUM")
    for kt_i in range(NSKV):
        pss = ps_st.tile([P, SQ], F32, name="pss_st", tag="pss_st")
        for dit in range(ND):
            lhsT = kT[dit][:, kt_i * P : (kt_i + 1) * P]
            for c in range(NC):
                nc.tensor.matmul(
                    pss[:, c * 512 : (c + 1) * 512],
                    lhsT,
                    qT[dit][:, c * 512 : (c + 1) * 512],
                    start=(dit == 0),
                    stop=(dit == ND - 1),
                )
        for c in range(NC):
            nc.scalar.activation(
                pT[kt_i][:, c * 512 : (c + 1) * 512],
                pss[:, c * 512 : (c + 1) * 512],
                AF.Exp,
                scale=0.125,
            )
    ps_st.release()

    ps_pv = tc.alloc_tile_pool(name="ps_pv", bufs=2, space="PSUM")
    ps_sum = tc.alloc_tile_pool(name="ps_sum", bufs=2, space="PSUM")
    for sqt in range(NSQ):
        pso = ps_pv.tile([P, D], F32, name="pso", tag="pso")
        psum_s = ps_sum.tile([P, 1], F32, name="psum_s", tag="psum_s")
        for kt_i in range(NSKV):
            lhsT = pT[kt_i][:, sqt * P : (sqt + 1) * P]
            first = kt_i == 0
            last = kt_i == NSKV - 1
            for h in range(2):
                nc.tensor.matmul(
                    pso[:, h * 512 : (h + 1) * 512],
                    lhsT,
                    vt[kt_i][:, h * 512 : (h + 1) * 512],
                    start=first,
                    stop=last,
                )
            nc.tensor.matmul(psum_s[:], lhsT, ones_col[:], start=first, stop=last)
        recip = small.tile([P, 1], F32, name="recip", tag="recip")
        nc.vector.reciprocal(recip[:], psum_s[:])
        ot = outp.tile([P, D], F32, name="ot", tag="ot")
        nc.vector.tensor_scalar_mul(ot[:], pso[:], recip[:])
        nc.sync.dma_start(out.ap()[sqt * P : (sqt + 1) * P, :], ot[:])

    ps_sum.release()
    ps_pv.release()
    small.release()
    outp.release()
    pt_pool.release()
    v_pool.release()
    kt_pool.release()
    qt_pool.release()
    const.release()


_NC_CACHE = {}


def _get_nc(fast):
    if fast not in _NC_CACHE:
        _NC_CACHE[fast] = build(fast=fast)
    return _NC_CACHE[fast]


def kernel(
    hidden_states,
    encoder_hidden_states,
    Wq,
    bq,
    Wk,
    bk,
    Wv,
    bv,
    _trace=False,
    _trace_kwargs=None,
):
    hs = np.ascontiguousarray(np.asarray(hidden_states, np.float32))
    es = np.ascontiguousarray(np.asarray(encoder_hidden_states, np.float32))
    wq_ = np.ascontiguousarray(np.asarray(Wq, np.float32))
    wk_ = np.ascontiguousarray(np.asarray(Wk, np.float32))
    wv_ = np.ascontiguousarray(np.asarray(Wv, np.float32))
    bq_ = np.ascontiguousarray(np.asarray(bq, np.float32))
    bk_ = np.ascontiguousarray(np.asarray(bk, np.float32))
    bv_ = np.ascontiguousarray(np.asarray(bv, np.float32))

    # The S = x (Wq Wk^T) e^T association only absorbs the biases when they
    # are zero; fall back to the general module otherwise.
    fast = not (bq_.any() or bk_.any() or bv_.any())
    nc = _get_nc(fast)
    in_maps = [
        {
            "x": hs[c],
            "e": es[c],
            "wq": wq_,
            "wk": wk_,
            "wv": wv_,
            "bq": bq_,
            "bk": bk_,
            "bv": bv_,
        }
        for c in range(N_CORES)
    ]
    res = run_bass_kernel_spmd(
        nc,
        in_maps,
        list(range(N_CORES)),
        trace=_trace,
        **(_trace_kwargs or {}),
    )
    out = np.stack([res.results[c]["out"] for c in range(N_CORES)], axis=0)
    if _trace:
        return out, res
    return out
